# revision 1
# baseline (speedup 1.0000x reference)
"""Trainium2 Bass kernel for nn_MDCN (mixture-density head forward pass).

Reference computation (B=2048, F=1024, M=128):
    rho = tanh(feature @ h2rho_w.T + h2rho_b);  rho[:, 0] = 0.95
    pi  = softmax(feature @ h2pi_w.T + h2pi_b)
    var0 = exp(feature @ h2var_w.T + h2var_b)
    var = (1 - exp(rho)) * var0 + 1e-4
    W_ = r*muW + s*(r*(zstd/wstd)*(W-muW) + Z*s),  s = sqrt(1-r^2)
    mu = einsum('bmf,bf->bm', W_, feature)

Key algebraic collapse: with a = (zstd/wstd)*(W-muW),
    mu[b,m] = r*d1[b] + r*s*d2[b] + s^2*d3[b]
  where d1 = feature@muW, d2 = feature@a, d3 = feature@Z.
So the [B,M,F] einsum becomes 3 extra columns of one fused matmul:
    logits[b, 0:387] = feature[b] @ [wrho.T | wpi.T | wvar.T | muW | a | Z]
Additionally s = sqrt(1-r^2) = sech(u) = (1+tanh(u)) * exp(-u), so the whole
epilogue needs only Tanh and Exp (one ACT table set), and the clamped first
mixture column is a compile-time constant.

Sharding: pure data-parallel over batch across 8 cores (256 rows/core),
weights replicated. No collectives needed (forward only).
"""

import os
from contextlib import ExitStack

import numpy as np

import concourse.bass as bass
import concourse.bacc as bacc
import concourse.mybir as mybir
import concourse.tile as tile
from concourse.bass_utils import run_bass_kernel_spmd

B, F, M = 2048, 1024, 128
NCORES = 8
BC = B // NCORES            # 256 batch rows per core
NT = BC // 128              # 2 partition tiles per core
KC = F // 128               # 8 contraction chunks
NW = 3 * M + 4              # 388 fused output columns (384 logits +
                            # 3 mu dot-products + 1 pad; fp32r matmul
                            # requires an even destination free-dim)
RHO_1 = np.float32(0.95)
TAU_INV = 1.0e-4
# s at the clamped column, computed exactly as the fp32 reference does:
# s0 = sqrt(1 - 0.95f * 0.95f)
S0 = float(np.sqrt(np.float32(1.0) - RHO_1 * RHO_1))

F32 = mybir.dt.float32
F32R = mybir.dt.float32r
F16 = mybir.dt.float16
AF = mybir.ActivationFunctionType
OP = mybir.AluOpType

# Matmul operand dtype. The per-core DMA path sustains only ~200 GB/s, so
# the kernel is input-bandwidth-bound and fp16 inputs halve its runtime.
# fp16 (11-bit mantissa) keeps the worst-case output error ~1e-3 of scale
# (vs 2.8e-4 for float32r, 4e-3 for bfloat16); accumulation is fp32 in PSUM.
# Set to F32R for a full-precision fallback (bit-compatible with fp32 but
# streams 1 row/cycle vs 4 for plain fp32).
MM_DT = F16
MM_NP = np.float16 if MM_DT == F16 else np.float32


def _emit_body(nc, tc, pools, fwc_dram, ft1_dram, blk_dram, out_dram):
    """Emit one full forward pass: DMA in -> fused matmul -> epilogue -> out."""
    consts, fwpool, psum, work = pools

    # Tiny bias block goes on the gpsimd (SWDGE) queue so it does not block
    # the head of the SP (HWDGE) queue that streams the big inputs.
    blk = consts.tile([1, 128 + NW], MM_DT, tag="bias_blk", name="bias_blk")
    nc.gpsimd.dma_start(blk[:], blk_dram)

    # Each dma_start carries ~0.5-2us of fixed cost on this part, so inputs
    # are consolidated into three transfers: two fused feature+weight halves
    # (PE starts on the first while the second streams) and tile-1 features.
    H = KC // 2
    fwcA = fwpool.tile([128, H, 128 + NW], MM_DT, tag="fwcA", name="fwcA")
    nc.sync.dma_start(fwcA[:], fwc_dram[0:H].rearrange("c p j -> p c j"))
    fwcB = fwpool.tile([128, H, 128 + NW], MM_DT, tag="fwcB", name="fwcB")
    nc.sync.dma_start(fwcB[:], fwc_dram[H:KC].rearrange("c p j -> p c j"))
    ft1 = fwpool.tile([128, KC, 128], MM_DT, tag="ft1", name="ft1")
    nc.sync.dma_start(ft1[:], ft1_dram)

    def fwc_c(c):
        return fwcA[:, c, :] if c < H else fwcB[:, c - H, :]

    # Fused matmul: psum[t][b, :] = bias + sum_c featT_c[:,b].T @ wcat_c
    pt = [psum.tile([128, NW], F32, tag=f"psum{t}", name=f"psum{t}")
          for t in range(NT)]
    for t in range(NT):
        nc.tensor.matmul(pt[t][:], blk[:, 0:128], blk[:, 128:128 + NW],
                         start=True, stop=False)
    for c in range(KC):
        nc.tensor.matmul(pt[0][:], fwc_c(c)[:, 0:128],
                         fwc_c(c)[:, 128:128 + NW],
                         start=False, stop=(c == KC - 1))
    for c in range(KC):
        nc.tensor.matmul(pt[1][:], ft1[:, c, :],
                         fwc_c(c)[:, 128:128 + NW],
                         start=False, stop=(c == KC - 1))

    # Epilogue per 128-row tile. Layout of psum P: [rho | pi | var | d1 d2 d3 0]
    for t in range(NT):
        P = pt[t][:]
        tg = f"t{t}"

        # psum layout (rho weights negated on host): [-u | pi | var | d 0].
        # r = tanh(u) = tanh(-1 * P[:,0:M]); one exp covers e^-u, e^pi, e^var.
        r = work.tile([128, M], F32, tag="r" + tg, name="r" + tg)
        nc.scalar.activation(r[:], P[:, 0:M], AF.Tanh, scale=-1.0)
        E = work.tile([128, 3 * M], F32, tag="E" + tg, name="E" + tg)
        nc.scalar.activation(E[:], P[:, 0:3 * M], AF.Exp)
        eneg, epi, var0 = E[:, 0:M], E[:, M:2 * M], E[:, 2 * M:3 * M]

        dsb = work.tile([128, 3], F32, tag="dsb" + tg, name="dsb" + tg)
        nc.vector.tensor_copy(dsb[:], P[:, 3 * M:3 * M + 3])

        # clamp first mixture BEFORE exp(rho) and the mu chain
        nc.vector.memset(r[:, 0:1], float(RHO_1))
        erho = work.tile([128, M], F32, tag="erho" + tg, name="erho" + tg)
        nc.scalar.activation(erho[:], r[:], AF.Exp)

        out_sb = work.tile([128, 3 * M], F32, tag="out" + tg, name="out" + tg)

        # s = (1 + r) * exp(-u) = sqrt(1 - r^2); fix clamped column
        s = work.tile([128, M], F32, tag="s" + tg, name="s" + tg)
        nc.vector.scalar_tensor_tensor(s[:], r[:], 1.0, eneg, OP.add, OP.mult)
        nc.vector.memset(s[:, 0:1], S0)

        # mu = r*(d1 + s*d2) + s^2*d3
        ss = work.tile([128, M], F32, tag="ss" + tg, name="ss" + tg)
        nc.vector.tensor_mul(ss[:], s[:], s[:])
        q = work.tile([128, M], F32, tag="q" + tg, name="q" + tg)
        nc.scalar.activation(q[:], s[:], AF.Identity,
                             bias=dsb[:, 0:1], scale=dsb[:, 1:2])
        rq = work.tile([128, M], F32, tag="rq" + tg, name="rq" + tg)
        nc.vector.tensor_mul(rq[:], r[:], q[:])
        nc.vector.scalar_tensor_tensor(out_sb[:, M:2 * M], ss[:], dsb[:, 2:3],
                                       rq[:], OP.mult, OP.add)

        # var = (1 - erho) * var0 + tau = -((erho - 1) * var0) + tau
        t1 = work.tile([128, M], F32, tag="t1" + tg, name="t1" + tg)
        nc.vector.scalar_tensor_tensor(t1[:], erho[:], 1.0, var0, OP.subtract,
                                       OP.mult)
        nc.vector.tensor_scalar(out_sb[:, 2 * M:3 * M], t1[:], -1.0, TAU_INV,
                                OP.mult, OP.add)

        # pi = epi / sum(epi)
        ssum = work.tile([128, 1], F32, tag="ssum" + tg, name="ssum" + tg)
        nc.vector.tensor_reduce(ssum[:], epi, mybir.AxisListType.X, OP.add)
        rsum = work.tile([128, 1], F32, tag="rsum" + tg, name="rsum" + tg)
        nc.vector.reciprocal(rsum[:], ssum[:])
        nc.vector.tensor_scalar_mul(out_sb[:, 0:M], epi, rsum[:])

        nc.sync.dma_start(out_dram[t * 128:(t + 1) * 128, :], out_sb[:])


def _declare_io(nc):
    # fwc: per contraction chunk, batch-tile-0 features fused with the weight
    # block (one DMA -> one matmul wait, and tile 0's inputs finish ~0.5MB of
    # DMA earlier than tile 1's, so its epilogue overlaps tile 1's loads).
    # ft1: tile-1 features, loaded last as two contiguous-run DMAs.
    fwc_dram = nc.dram_tensor("fwc", [KC, 128, 128 + NW], MM_DT,
                              kind="ExternalInput").ap()
    ft1_dram = nc.dram_tensor("ft1", [128, KC, 128], MM_DT,
                              kind="ExternalInput").ap()
    blk_dram = nc.dram_tensor("bias_blk", [1, 128 + NW], MM_DT,
                              kind="ExternalInput").ap()
    out_dram = nc.dram_tensor("out", [BC, 3 * M], F32, kind="ExternalOutput").ap()
    return fwc_dram, ft1_dram, blk_dram, out_dram


def _warmup_act(nc, consts):
    # Trigger the ACT exp/tanh table load immediately, overlapping the
    # input DMAs (it costs ~2.7us once per kernel).
    warm_in = consts.tile([128, 1], F32, tag="warm_in", name="warm_in")
    warm_out = consts.tile([128, 1], F32, tag="warm_out", name="warm_out")
    nc.vector.memset(warm_in[:], 0.0)
    nc.scalar.activation(warm_out[:], warm_in[:], AF.Exp)


def _warmup_pe(nc, consts, psum, n_fillers=9):
    # The PE HAM clock-gate only unthrottles (1.2 -> 2.4 GHz) after ~3.4us of
    # sustained activity. Feed it scratch matmuls while the input DMAs stream
    # so the real (dependency-gated) matmuls run at full clock.
    wsrc = consts.tile([1, 128], MM_DT, tag="pe_w", name="pe_w")
    nc.vector.memset(wsrc[:], 1.0)
    msrc = consts.tile([1, 512], MM_DT, tag="pe_m", name="pe_m")
    nc.vector.memset(msrc[:], 1.0)
    scratch = psum.tile([128, 512], F32, tag="pe_scratch", name="pe_scratch",
                        bufs=1)
    for i in range(n_fillers):
        nc.tensor.matmul(scratch[:], wsrc[:], msrc[:], start=True, stop=True)


def _build_nc():
    nc = bacc.Bacc("TRN2", target_bir_lowering=False, debug=False)
    fwc_dram, ft1_dram, blk_dram, out_dram = _declare_io(nc)
    with tile.TileContext(nc) as tc, ExitStack() as ctx:
        consts = ctx.enter_context(tc.tile_pool(name="consts", bufs=1))
        fwpool = ctx.enter_context(tc.tile_pool(name="fw", bufs=1))
        psum = ctx.enter_context(tc.tile_pool(name="psum", bufs=NT, space="PSUM"))
        work = ctx.enter_context(tc.tile_pool(name="work", bufs=NT))
        _warmup_act(nc, consts)
        _warmup_pe(nc, consts, psum)
        _emit_body(nc, tc, (consts, fwpool, psum, work),
                   fwc_dram, ft1_dram, blk_dram, out_dram)
    nc.compile()
    return nc


def build_loop_nc(reps):
    """Timing variant: run the body `reps` times inside one NEFF (used only
    by the local test harness; the default full-barrier back-edge keeps
    iterations serialized so per-iter span ~ single-shot kernel time)."""
    nc = bacc.Bacc("TRN2", target_bir_lowering=False, debug=False)
    fwc_dram, ft1_dram, blk_dram, out_dram = _declare_io(nc)
    with tile.TileContext(nc) as tc, ExitStack() as ctx:
        consts = ctx.enter_context(tc.tile_pool(name="consts", bufs=1))
        fwpool = ctx.enter_context(tc.tile_pool(name="fw", bufs=1))
        psum = ctx.enter_context(tc.tile_pool(name="psum", bufs=NT, space="PSUM"))
        work = ctx.enter_context(tc.tile_pool(name="work", bufs=NT))
        _warmup_act(nc, consts)
        with tc.For_i(0, reps, 1):
            _warmup_pe(nc, consts, psum)
            _emit_body(nc, tc, (consts, fwpool, psum, work),
                       fwc_dram, ft1_dram, blk_dram, out_dram)
    nc.compile()
    return nc


_CACHE = {}


def _get_nc():
    if "nc" not in _CACHE:
        _CACHE["nc"] = _build_nc()
    return _CACHE["nc"]


def _host_prep(inputs):
    f32 = np.float32
    feature = np.ascontiguousarray(inputs["feature"], dtype=f32)
    muW = np.asarray(inputs["muW"], dtype=f32)
    W = np.asarray(inputs["W"], dtype=f32)
    Z = np.asarray(inputs["Z"], dtype=f32)
    logvarW = np.asarray(inputs["logvarW"], dtype=f32)
    logvarZ = np.asarray(inputs["logvarZ"], dtype=f32)

    wstd = np.sqrt(np.exp(logvarW)).astype(f32)
    zstd = np.sqrt(np.exp(logvarZ)).astype(f32)
    a = ((zstd / wstd).astype(f32) * (W - muW)).astype(f32)
    v3 = np.stack([muW, a, Z, np.zeros_like(muW)], axis=1)  # [F, 4]

    wcat = np.concatenate(
        [-np.asarray(inputs["h2rho_w"], dtype=f32).T,
         np.asarray(inputs["h2pi_w"], dtype=f32).T,
         np.asarray(inputs["h2var_w"], dtype=f32).T,
         v3],
        axis=1,
    )  # [F, 387]
    wcat = wcat.reshape(KC, 128, NW)

    bias_blk = np.concatenate(
        [np.ones(128, dtype=f32),
         -np.asarray(inputs["h2rho_b"], dtype=f32),
         np.asarray(inputs["h2pi_b"], dtype=f32),
         np.asarray(inputs["h2var_b"], dtype=f32),
         np.zeros(4, dtype=f32)],
    ).reshape(1, 128 + NW)
    bias_blk = np.ascontiguousarray(bias_blk)

    in_maps = []
    for c in range(NCORES):
        shard = feature[c * BC:(c + 1) * BC]            # [BC, F]
        featT = shard.T.reshape(KC, 128, NT, 128)       # [c, p, half, j]
        fwc = np.ascontiguousarray(
            np.concatenate([featT[:, :, 0, :], wcat], axis=2),
            dtype=MM_NP)                                # [KC,128,128+NW]
        ft1 = np.ascontiguousarray(
            featT[:, :, 1, :].transpose(1, 0, 2), dtype=MM_NP)  # [128(p),KC,128]
        in_maps.append({"fwc": fwc, "ft1": ft1,
                        "bias_blk": bias_blk.astype(MM_NP)})
    return in_maps


def kernel(**inputs):
    nc = _get_nc()
    in_maps = _host_prep(inputs)
    res = run_bass_kernel_spmd(nc, in_maps, list(range(NCORES)))
    full = np.concatenate([res.results[c]["out"] for c in range(NCORES)], axis=0)
    pi = np.ascontiguousarray(full[:, 0:M])
    mu = np.ascontiguousarray(full[:, M:2 * M])
    var = np.ascontiguousarray(full[:, 2 * M:3 * M])
    return pi, mu, var



# revision 2
# speedup vs baseline: 1.0943x; 1.0943x over previous
"""Trainium2 Bass kernel for nn_MDCN (mixture-density head forward pass).

Reference computation (B=2048, F=1024, M=128):
    rho = tanh(feature @ h2rho_w.T + h2rho_b);  rho[:, 0] = 0.95
    pi  = softmax(feature @ h2pi_w.T + h2pi_b)
    var0 = exp(feature @ h2var_w.T + h2var_b)
    var = (1 - exp(rho)) * var0 + 1e-4
    W_ = r*muW + s*(r*(zstd/wstd)*(W-muW) + Z*s),  s = sqrt(1-r^2)
    mu = einsum('bmf,bf->bm', W_, feature)

Algebraic collapse (as before): with a = (zstd/wstd)*(W-muW),
    mu[b,m] = r*d1[b] + r*s*d2[b] + s^2*d3[b]
  where d1 = feature@muW, d2 = feature@a, d3 = feature@Z ride as 3 extra
matmul columns. s = sqrt(1-r^2) = (1+tanh u) * exp(-u) needs only Tanh+Exp.
The rho[:,0]=0.95 clamp is folded into the weights: rho weight column 0 is
zeroed and its bias set to atanh(0.95), so column 0 computes the constant
with no epilogue special-casing.

v2 structure (the previous version was fully serial: 6.4us DMA + 4.7us
matmul + 4.6us epilogue + 3.6us output = 20us; measured per-core DMA tops
out at ~220 GB/s no matter how many queues, so everything must hide under
the input stream):
  - Inputs stream on one HWDGE queue in arrival order: features first,
    then weights in three column groups: pi -> rho+d -> var.
  - PE chases the stream (filler matmuls keep the PE p-state ramping
    while features load).
  - Each head's epilogue runs as soon as its group's psum closes, hiding
    under the remaining weight stream; only the var tail (exp + 2 DVE
    ops) lands after the stream.
  - Epilogue ops are stacked across the two 128-row batch tiles via
    strided PSUM access patterns (psum tiles are [128, 2, 512] = 2
    banks), halving instruction-fixed costs. fp16 intermediates double
    DVE throughput. Softmax sums ride the exp via accum_out.
  - One fp16 output DMA (197KB instead of 2x fp32 = 393KB + extra fixed
    costs); host upcasts.

Sharding: pure data-parallel over batch across 8 cores (256 rows/core),
weights replicated. No collectives (forward only).
"""

from contextlib import ExitStack

import numpy as np

import concourse.bass as bass
import concourse.bacc as bacc
import concourse.mybir as mybir
import concourse.tile as tile
from concourse.bass_utils import run_bass_kernel_spmd

B, F, M = 2048, 1024, 128
NCORES = 8
BC = B // NCORES            # 256 batch rows per core
NT = BC // 128              # 2 partition tiles per core
KC = F // 128               # 8 contraction chunks
GW_RHOD = M + 4             # rho group: 128 rho cols + d1,d2,d3 + pad
RHO_1 = np.float32(0.95)
TAU_INV = 1.0e-4
U0 = float(np.arctanh(np.float32(0.95)))   # folded rho[:,0] clamp

F32 = mybir.dt.float32
F16 = mybir.dt.float16
AF = mybir.ActivationFunctionType
OP = mybir.AluOpType

MM_NP = np.float16
N_FILL = 5                  # PE p-state warmup fillers per iteration


def _declare_io(nc):
    # ft[p, c, t, b] = feature[t*128+b, c*128+p]; stationary operand.
    ft_dram = nc.dram_tensor("ft", [128, KC, NT, 128], F16,
                             kind="ExternalInput").ap()
    wpi_dram = nc.dram_tensor("wpi", [128, KC, M], F16,
                              kind="ExternalInput").ap()
    wrhod_dram = nc.dram_tensor("wrhod", [128, KC, GW_RHOD], F16,
                                kind="ExternalInput").ap()
    wvar_dram = nc.dram_tensor("wvar", [128, KC, M], F16,
                               kind="ExternalInput").ap()
    # blk: [ones(128) | bias_pi(128) | bias_rhod(132) | bias_var(128)]
    blk_dram = nc.dram_tensor("bias_blk", [1, 128 + M + GW_RHOD + M], F16,
                              kind="ExternalInput").ap()
    out_dram = nc.dram_tensor("out", [NT, 128, 3 * M], F16,
                              kind="ExternalOutput").ap()
    return ft_dram, wpi_dram, wrhod_dram, wvar_dram, blk_dram, out_dram


def _warmup_act(nc, consts):
    # Trigger the ACT exp/tanh table load immediately (costs ~2.7us once).
    warm_in = consts.tile([128, 1], F32, tag="warm_in", name="warm_in")
    warm_out = consts.tile([128, 1], F32, tag="warm_out", name="warm_out")
    nc.vector.memset(warm_in[:], 0.0)
    nc.scalar.activation(warm_out[:], warm_in[:], AF.Exp)


def _warmup_pe(nc, consts, scratch):
    # Keep the PE busy while the feature stream lands so the p-state ramp
    # (0.65 -> 1.2 -> 2.4 GHz after ~3us of sustained activity) happens
    # before the real matmuls run.
    wsrc = consts.tile([1, 128], F16, tag="pe_w", name="pe_w")
    nc.vector.memset(wsrc[:], 1.0)
    msrc = consts.tile([1, 512], F16, tag="pe_m", name="pe_m")
    nc.vector.memset(msrc[:], 1.0)
    for _ in range(N_FILL):
        nc.tensor.matmul(scratch[:], wsrc[:], msrc[:], start=True, stop=True)


def _emit_body(nc, tc, pools, drams):
    consts, fwpool, psum, work = pools
    ft_dram, wpi_dram, wrhod_dram, wvar_dram, blk_dram, out_dram = drams

    BW = 128 + M + GW_RHOD + M
    blk = consts.tile([1, BW], F16, tag="bias_blk", name="bias_blk")
    nc.gpsimd.dma_start(blk[:], blk_dram)

    # Input stream, arrival order = use order.
    ft = fwpool.tile([128, KC, NT, 128], F16, tag="ft", name="ft")
    nc.sync.dma_start(ft[:], ft_dram)
    wpi = fwpool.tile([128, KC, M], F16, tag="wpi", name="wpi")
    nc.sync.dma_start(wpi[:], wpi_dram)
    wrhod = fwpool.tile([128, KC, GW_RHOD], F16, tag="wrhod", name="wrhod")
    nc.sync.dma_start(wrhod[:], wrhod_dram)
    wvar = fwpool.tile([128, KC, M], F16, tag="wvar", name="wvar")
    nc.sync.dma_start(wvar[:], wvar_dram)

    # PSUM: one [128, 2, 512] tile (2 banks) per column group; matmul
    # dests are the per-tile halves, epilogue reads stacked strided APs.
    P_pi = psum.tile([128, NT, 512], F32, tag="P_pi", name="P_pi")
    P_rhod = psum.tile([128, NT, 512], F32, tag="P_rhod", name="P_rhod")
    P_var = psum.tile([128, NT, 512], F32, tag="P_var", name="P_var")

    b_pi = blk[:, 128:128 + M]
    b_rhod = blk[:, 128 + M:128 + M + GW_RHOD]
    b_var = blk[:, 128 + M + GW_RHOD:BW]
    ones = blk[:, 0:128]
    for t in range(NT):
        nc.tensor.matmul(P_pi[:, t, 0:M], ones, b_pi, start=True, stop=False)
        nc.tensor.matmul(P_rhod[:, t, 0:GW_RHOD], ones, b_rhod,
                         start=True, stop=False)
        nc.tensor.matmul(P_var[:, t, 0:M], ones, b_var, start=True, stop=False)
    for g, (Pg, wg, gw) in enumerate([(P_pi, wpi, M), (P_rhod, wrhod, GW_RHOD),
                                      (P_var, wvar, M)]):
        for c in range(KC):
            for t in range(NT):
                nc.tensor.matmul(Pg[:, t, 0:gw], ft[:, c, t, :], wg[:, c, :],
                                 start=False, stop=(c == KC - 1))

    out_sb = work.tile([128, NT, 3 * M], F16, tag="out_sb", name="out_sb")

    # --- pi phase: softmax, sum rides the exp via accum_out ---
    e_pi = work.tile([128, NT, M], F16, tag="e_pi", name="e_pi")
    ssum = work.tile([128, NT], F32, tag="ssum", name="ssum")
    for t in range(NT):
        nc.scalar.activation(e_pi[:, t, :], P_pi[:, t, 0:M], AF.Exp,
                             accum_out=ssum[:, t:t + 1])
    rsum = work.tile([128, NT], F32, tag="rsum", name="rsum")
    nc.vector.reciprocal(rsum[:], ssum[:])
    for t in range(NT):
        nc.vector.tensor_scalar_mul(out_sb[:, t, 0:M], e_pi[:, t, :],
                                    rsum[:, t:t + 1])

    # --- rho phase: r, s, mu ---
    r = work.tile([128, NT, M], F16, tag="r", name="r")
    nc.scalar.activation(r[:], P_rhod[:, :, 0:M], AF.Tanh, scale=-1.0)
    eneg = work.tile([128, NT, M], F16, tag="eneg", name="eneg")
    nc.scalar.activation(eneg[:], P_rhod[:, :, 0:M], AF.Exp)
    erho = work.tile([128, NT, M], F16, tag="erho", name="erho")
    nc.scalar.activation(erho[:], r[:], AF.Exp)

    s = work.tile([128, NT, M], F16, tag="s", name="s")
    nc.vector.scalar_tensor_tensor(s[:], r[:], 1.0, eneg[:], OP.add, OP.mult)
    ss = work.tile([128, NT, M], F16, tag="ss", name="ss")
    nc.gpsimd.tensor_mul(ss[:], s[:], s[:])
    q = work.tile([128, NT, M], F16, tag="q", name="q")
    for t in range(NT):
        # q = d1 + s*d2; d-scalars read straight out of PSUM
        nc.vector.tensor_scalar(q[:, t, :], s[:, t, :],
                                P_rhod[:, t, M + 1:M + 2],
                                P_rhod[:, t, M:M + 1], OP.mult, OP.add)
    rq = work.tile([128, NT, M], F16, tag="rq", name="rq")
    nc.vector.tensor_mul(rq[:], r[:], q[:])
    for t in range(NT):
        # mu = ss*d3 + rq
        nc.vector.scalar_tensor_tensor(out_sb[:, t, M:2 * M], ss[:, t, :],
                                       P_rhod[:, t, M + 2:M + 3], rq[:, t, :],
                                       OP.mult, OP.add)

    # --- var phase: var = -(erho-1)*var0 + tau ---
    ev = work.tile([128, NT, M], F16, tag="ev", name="ev")
    nc.scalar.activation(ev[:], P_var[:, :, 0:M], AF.Exp)
    t1 = work.tile([128, NT, M], F16, tag="t1", name="t1")
    nc.vector.scalar_tensor_tensor(t1[:], erho[:], 1.0, ev[:], OP.subtract,
                                   OP.mult)
    nc.vector.tensor_scalar(out_sb[:, :, 2 * M:3 * M], t1[:], -1.0, TAU_INV,
                            OP.mult, OP.add)

    nc.sync.dma_start(out_dram.rearrange("t p j -> p t j"), out_sb[:])


def _build_pools(tc, ctx):
    consts = ctx.enter_context(tc.tile_pool(name="consts", bufs=1))
    fwpool = ctx.enter_context(tc.tile_pool(name="fw", bufs=1))
    psum = ctx.enter_context(tc.tile_pool(name="psum", bufs=1, space="PSUM"))
    work = ctx.enter_context(tc.tile_pool(name="work", bufs=1))
    return consts, fwpool, psum, work


def _build_nc():
    nc = bacc.Bacc("TRN2", target_bir_lowering=False, debug=False)
    drams = _declare_io(nc)
    with tile.TileContext(nc) as tc, ExitStack() as ctx:
        consts, fwpool, psum, work = _build_pools(tc, ctx)
        scratch = psum.tile([128, 512], F32, tag="pe_scratch",
                            name="pe_scratch")
        _warmup_act(nc, consts)
        _warmup_pe(nc, consts, scratch)
        _emit_body(nc, tc, (consts, fwpool, psum, work), drams)
    nc.compile()
    return nc


def build_loop_nc(reps):
    """Timing variant: run the body `reps` times inside one NEFF (used only
    by the local test harness; the default full-barrier back-edge keeps
    iterations serialized so per-iter span ~ single-shot kernel time)."""
    nc = bacc.Bacc("TRN2", target_bir_lowering=False, debug=False)
    drams = _declare_io(nc)
    with tile.TileContext(nc) as tc, ExitStack() as ctx:
        consts, fwpool, psum, work = _build_pools(tc, ctx)
        scratch = psum.tile([128, 512], F32, tag="pe_scratch",
                            name="pe_scratch")
        _warmup_act(nc, consts)
        with tc.For_i(0, reps, 1):
            _warmup_pe(nc, consts, scratch)
            _emit_body(nc, tc, (consts, fwpool, psum, work), drams)
    nc.compile()
    return nc


_CACHE = {}


def _get_nc():
    if "nc" not in _CACHE:
        _CACHE["nc"] = _build_nc()
    return _CACHE["nc"]


def _host_prep(inputs):
    f32 = np.float32
    feature = np.ascontiguousarray(inputs["feature"], dtype=f32)
    muW = np.asarray(inputs["muW"], dtype=f32)
    W = np.asarray(inputs["W"], dtype=f32)
    Z = np.asarray(inputs["Z"], dtype=f32)
    logvarW = np.asarray(inputs["logvarW"], dtype=f32)
    logvarZ = np.asarray(inputs["logvarZ"], dtype=f32)

    wstd = np.sqrt(np.exp(logvarW)).astype(f32)
    zstd = np.sqrt(np.exp(logvarZ)).astype(f32)
    a = ((zstd / wstd).astype(f32) * (W - muW)).astype(f32)

    # Column groups; rho weights negated so psum = -u and exp(psum) = e^-u.
    wpi = np.asarray(inputs["h2pi_w"], dtype=f32).T          # [F, M]
    wrho = -np.asarray(inputs["h2rho_w"], dtype=f32).T       # [F, M]
    wrho[:, 0] = 0.0                                         # folded clamp
    wvar = np.asarray(inputs["h2var_w"], dtype=f32).T        # [F, M]
    wrhod = np.concatenate(
        [wrho, np.stack([muW, a, Z, np.zeros_like(muW)], axis=1)], axis=1)

    b_pi = np.asarray(inputs["h2pi_b"], dtype=f32)
    b_rho = -np.asarray(inputs["h2rho_b"], dtype=f32)
    b_rho[0] = -U0                                           # folded clamp
    b_var = np.asarray(inputs["h2var_b"], dtype=f32)
    blk = np.concatenate(
        [np.ones(128, dtype=f32), b_pi, b_rho, np.zeros(4, dtype=f32),
         b_var]).reshape(1, -1).astype(MM_NP)

    # [F, gw] -> [128(p), KC, gw]
    def wfmt(w):
        return np.ascontiguousarray(
            w.reshape(KC, 128, w.shape[1]).transpose(1, 0, 2), dtype=MM_NP)

    wpi_h, wrhod_h, wvar_h = wfmt(wpi), wfmt(wrhod), wfmt(wvar)

    in_maps = []
    for cr in range(NCORES):
        shard = feature[cr * BC:(cr + 1) * BC]               # [BC, F]
        # ft[p, c, t, b] = shard[t*128+b, c*128+p]
        ft = np.ascontiguousarray(
            shard.reshape(NT, 128, KC, 128).transpose(3, 2, 0, 1),
            dtype=MM_NP)
        in_maps.append({"ft": ft, "wpi": wpi_h, "wrhod": wrhod_h,
                        "wvar": wvar_h, "bias_blk": blk})
    return in_maps


def _postprocess(res, cores):
    full = np.concatenate(
        [np.asarray(res.results[c]["out"], dtype=np.float32).reshape(BC, 3 * M)
         for c in cores], axis=0)
    pi = np.ascontiguousarray(full[:, 0:M])
    mu = np.ascontiguousarray(full[:, M:2 * M])
    var = np.ascontiguousarray(full[:, 2 * M:3 * M])
    return pi, mu, var


def kernel(**inputs):
    nc = _get_nc()
    in_maps = _host_prep(inputs)
    res = run_bass_kernel_spmd(nc, in_maps, list(range(NCORES)))
    return _postprocess(res, list(range(NCORES)))


# revision 15
# speedup vs baseline: 1.1622x; 1.0620x over previous
"""Trainium2 Bass kernel for nn_MDCN (mixture-density head forward pass).

Reference computation (B=2048, F=1024, M=128):
    rho = tanh(feature @ h2rho_w.T + h2rho_b);  rho[:, 0] = 0.95
    pi  = softmax(feature @ h2pi_w.T + h2pi_b)
    var0 = exp(feature @ h2var_w.T + h2var_b)
    var = (1 - exp(rho)) * var0 + 1e-4
    W_ = r*muW + s*(r*(zstd/wstd)*(W-muW) + Z*s),  s = sqrt(1-r^2)
    mu = einsum('bmf,bf->bm', W_, feature)

Algebraic collapse (as before): with a = (zstd/wstd)*(W-muW),
    mu[b,m] = r*d1[b] + r*s*d2[b] + s^2*d3[b]
  where d1 = feature@muW, d2 = feature@a, d3 = feature@Z ride as 3 extra
matmul columns. s = sqrt(1-r^2) = (1+tanh u) * exp(-u) needs only Tanh+Exp.
The rho[:,0]=0.95 clamp is folded into the weights: rho weight column 0 is
zeroed and its bias set to atanh(0.95), so column 0 computes the constant
with no epilogue special-casing.

v2 structure (the previous version was fully serial: 6.4us DMA + 4.7us
matmul + 4.6us epilogue + 3.6us output = 20us; measured per-core DMA tops
out at ~220 GB/s no matter how many queues, so everything must hide under
the input stream):
  - Inputs stream on one HWDGE queue in arrival order: features first,
    then weights in three column groups: pi -> rho+d -> var.
  - PE chases the stream (filler matmuls keep the PE p-state ramping
    while features load).
  - Each head's epilogue runs as soon as its group's psum closes, hiding
    under the remaining weight stream; only the var tail (exp + 2 DVE
    ops) lands after the stream.
  - Epilogue ops are stacked across the two 128-row batch tiles via
    strided PSUM access patterns (psum tiles are [128, 2, 512] = 2
    banks), halving instruction-fixed costs. fp16 intermediates double
    DVE throughput. Softmax sums ride the exp via accum_out.
  - One fp16 output DMA (197KB instead of 2x fp32 = 393KB + extra fixed
    costs); host upcasts.

Sharding: pure data-parallel over batch across 8 cores (256 rows/core),
weights replicated. No collectives (forward only).
"""

from contextlib import ExitStack

import numpy as np

import concourse.bass as bass
import concourse.bacc as bacc
import concourse.mybir as mybir
import concourse.tile as tile
from concourse.bass_utils import run_bass_kernel_spmd

B, F, M = 2048, 1024, 128
NCORES = 8
BC = B // NCORES            # 256 batch rows per core
NT = BC // 128              # 2 partition tiles per core
KC = F // 128               # 8 contraction chunks
GW_RHOD = M + 4             # rho group: 128 rho cols + d1,d2,d3 + pad
RHO_1 = np.float32(0.95)
TAU_INV = 1.0e-4
U0 = float(np.arctanh(np.float32(0.95)))   # folded rho[:,0] clamp

F32 = mybir.dt.float32
F16 = mybir.dt.float16
AF = mybir.ActivationFunctionType
OP = mybir.AluOpType

MM_NP = np.float16
N_FILL = 9                  # PE p-state warmup fillers per iteration


def _declare_io(nc):
    # ft[p, c, t, b] = feature[t*128+b, c*128+p]; stationary operand.
    ft_dram = nc.dram_tensor("ft", [128, KC, NT, 128], F16,
                             kind="ExternalInput").ap()
    wpi_dram = nc.dram_tensor("wpi", [128, KC, M], F16,
                              kind="ExternalInput").ap()
    wrhod_dram = nc.dram_tensor("wrhod", [128, KC, GW_RHOD], F16,
                                kind="ExternalInput").ap()
    wvar_dram = nc.dram_tensor("wvar", [128, KC, M], F16,
                               kind="ExternalInput").ap()
    # blk: [ones(128) | bias_pi(128) | bias_rhod(132) | bias_var(128)]
    blk_dram = nc.dram_tensor("bias_blk", [1, 128 + M + GW_RHOD + M], F16,
                              kind="ExternalInput").ap()
    # Partition-major outputs (one per head): 512B-contiguous descriptors,
    # and each can ship as soon as its head finishes.
    opi_dram = nc.dram_tensor("out_pi", [128, NT * M], F16,
                              kind="ExternalOutput").ap()
    omu_dram = nc.dram_tensor("out_mu", [128, NT * M], F16,
                              kind="ExternalOutput").ap()
    ovar_dram = nc.dram_tensor("out_var", [128, NT * M], F16,
                               kind="ExternalOutput").ap()
    return (ft_dram, wpi_dram, wrhod_dram, wvar_dram, blk_dram,
            opi_dram, omu_dram, ovar_dram)


def _warmup_act(nc, consts):
    # Trigger the ACT exp/tanh table load immediately (costs ~2.7us once).
    warm_in = consts.tile([128, 1], F32, tag="warm_in", name="warm_in")
    warm_out = consts.tile([128, 1], F32, tag="warm_out", name="warm_out")
    nc.vector.memset(warm_in[:], 0.0)
    nc.scalar.activation(warm_out[:], warm_in[:], AF.Exp)


def _warmup_pe(nc, consts, scratch):
    # Keep the PE busy while the feature stream lands so the p-state ramp
    # (0.65 -> 1.2 -> 2.4 GHz after ~3us of sustained activity) happens
    # before the real matmuls run.
    wsrc = consts.tile([1, 128], F16, tag="pe_w", name="pe_w")
    nc.vector.memset(wsrc[:], 1.0)
    msrc = consts.tile([1, 512], F16, tag="pe_m", name="pe_m")
    nc.vector.memset(msrc[:], 1.0)
    for _ in range(N_FILL):
        nc.tensor.matmul(scratch[:], wsrc[:], msrc[:], start=True, stop=True)


def _emit_body(nc, tc, pools, drams, parts=("dma", "mm", "epi", "out")):
    consts, fwpool, psum, work = pools
    (ft_dram, wpi_dram, wrhod_dram, wvar_dram, blk_dram,
     opi_dram, omu_dram, ovar_dram) = drams

    BW = 128 + M + GW_RHOD + M
    blk = consts.tile([1, BW], F16, tag="bias_blk", name="bias_blk")
    nc.gpsimd.dma_start(blk[:], blk_dram)

    # Input stream, arrival order = use order: features, then weights in
    # column groups rho+d -> pi -> var. Each group's epilogue chain hides
    # under the remaining stream; only var's short chain trails it.
    ft = fwpool.tile([128, KC, NT, 128], F16, tag="ft", name="ft")
    nc.sync.dma_start(ft[:], ft_dram)
    wrhod = fwpool.tile([128, KC, GW_RHOD], F16, tag="wrhod", name="wrhod")
    nc.sync.dma_start(wrhod[:], wrhod_dram)
    wpi = fwpool.tile([128, KC, M], F16, tag="wpi", name="wpi")
    nc.sync.dma_start(wpi[:], wpi_dram)
    wvar = fwpool.tile([128, KC, M], F16, tag="wvar", name="wvar")
    nc.sync.dma_start(wvar[:], wvar_dram)
    if "mm" not in parts:
        return

    # PSUM: one [128, 2, 512] tile (2 banks) per column group; matmul
    # dests are the per-tile halves, epilogue reads stacked strided APs.
    P_pi = psum.tile([128, NT, 512], F32, tag="P_pi", name="P_pi")
    P_rhod = psum.tile([128, NT, 512], F32, tag="P_rhod", name="P_rhod")
    P_var = psum.tile([128, NT, 512], F32, tag="P_var", name="P_var")

    b_pi = blk[:, 128:128 + M]
    b_rhod = blk[:, 128 + M:128 + M + GW_RHOD]
    b_var = blk[:, 128 + M + GW_RHOD:BW]
    ones = blk[:, 0:128]
    for t in range(NT):
        nc.tensor.matmul(P_rhod[:, t, 0:GW_RHOD], ones, b_rhod,
                         start=True, stop=False)
        nc.tensor.matmul(P_pi[:, t, 0:M], ones, b_pi, start=True, stop=False)
        nc.tensor.matmul(P_var[:, t, 0:M], ones, b_var, start=True, stop=False)
    for Pg, wg, gw in [(P_rhod, wrhod, GW_RHOD), (P_pi, wpi, M),
                       (P_var, wvar, M)]:
        for c in range(KC):
            for t in range(NT):
                nc.tensor.matmul(Pg[:, t, 0:gw], ft[:, c, t, :], wg[:, c, :],
                                 start=False, stop=(c == KC - 1))

    if "epi" not in parts:
        return

    # Flat [128, NT*M] output tiles -> one 512B descriptor per partition.
    o_pi = work.tile([128, NT * M], F16, tag="o_pi", name="o_pi")
    o_mu = work.tile([128, NT * M], F16, tag="o_mu", name="o_mu")
    o_var = work.tile([128, NT * M], F16, tag="o_var", name="o_var")

    # --- rho phase: r, s, mu (starts while pi/var weights stream) ---
    r = work.tile([128, NT, M], F16, tag="r", name="r")
    nc.scalar.activation(r[:], P_rhod[:, :, 0:M], AF.Tanh, scale=-1.0)
    eneg = work.tile([128, NT, M], F16, tag="eneg", name="eneg")
    nc.scalar.activation(eneg[:], P_rhod[:, :, 0:M], AF.Exp)
    erho = work.tile([128, NT, M], F16, tag="erho", name="erho")
    nc.scalar.activation(erho[:], r[:], AF.Exp)

    s = work.tile([128, NT, M], F16, tag="s", name="s")
    nc.vector.scalar_tensor_tensor(s[:], r[:], 1.0, eneg[:], OP.add, OP.mult)
    ss = work.tile([128, NT, M], F16, tag="ss", name="ss")
    nc.gpsimd.tensor_mul(ss[:], s[:], s[:])
    q = work.tile([128, NT, M], F16, tag="q", name="q")
    for t in range(NT):
        # q = d1 + s*d2; d-scalars read straight out of PSUM
        nc.vector.tensor_scalar(q[:, t, :], s[:, t, :],
                                P_rhod[:, t, M + 1:M + 2],
                                P_rhod[:, t, M:M + 1], OP.mult, OP.add)
    rq = work.tile([128, NT, M], F16, tag="rq", name="rq")
    nc.vector.tensor_mul(rq[:], r[:], q[:])
    for t in range(NT):
        # mu = ss*d3 + rq
        nc.vector.scalar_tensor_tensor(o_mu[:, t * M:(t + 1) * M], ss[:, t, :],
                                       P_rhod[:, t, M + 2:M + 3], rq[:, t, :],
                                       OP.mult, OP.add)
    if "out" in parts:
        nc.gpsimd.dma_start(omu_dram, o_mu[:])

    # --- pi phase: softmax, sum rides the exp via accum_out ---
    e_pi = work.tile([128, NT, M], F16, tag="e_pi", name="e_pi")
    ssums, rsums = [], []
    for t in range(NT):
        ssums.append(work.tile([128, 1], F32, tag=f"ssum{t}",
                               name=f"ssum{t}"))
        rsums.append(work.tile([128, 1], F32, tag=f"rsum{t}",
                               name=f"rsum{t}"))
    for t in range(NT):
        nc.scalar.activation(e_pi[:, t, :], P_pi[:, t, 0:M], AF.Exp,
                             accum_out=ssums[t][:])
    for t in range(NT):
        nc.vector.reciprocal(rsums[t][:], ssums[t][:])
        nc.vector.tensor_scalar_mul(o_pi[:, t * M:(t + 1) * M], e_pi[:, t, :],
                                    rsums[t][:])
    if "out" in parts:
        nc.sync.dma_start(opi_dram, o_pi[:])

    # --- var phase (tail): var = -(erho-1)*var0 + tau ---
    ev = work.tile([128, NT, M], F16, tag="ev", name="ev")
    nc.scalar.activation(ev[:], P_var[:, :, 0:M], AF.Exp)
    t1 = work.tile([128, NT, M], F16, tag="t1", name="t1")
    nc.vector.scalar_tensor_tensor(t1[:], erho[:], 1.0, ev[:], OP.subtract,
                                   OP.mult)
    for t in range(NT):
        nc.vector.tensor_scalar(o_var[:, t * M:(t + 1) * M], t1[:, t, :],
                                -1.0, TAU_INV, OP.mult, OP.add)
    if "out" in parts:
        nc.scalar.dma_start(ovar_dram, o_var[:])


def _build_pools(tc, ctx):
    consts = ctx.enter_context(tc.tile_pool(name="consts", bufs=1))
    fwpool = ctx.enter_context(tc.tile_pool(name="fw", bufs=1))
    psum = ctx.enter_context(tc.tile_pool(name="psum", bufs=1, space="PSUM"))
    work = ctx.enter_context(tc.tile_pool(name="work", bufs=1))
    return consts, fwpool, psum, work


def _build_nc():
    nc = bacc.Bacc("TRN2", target_bir_lowering=False, debug=False)
    drams = _declare_io(nc)
    with tile.TileContext(nc) as tc, ExitStack() as ctx:
        consts, fwpool, psum, work = _build_pools(tc, ctx)
        scratch = psum.tile([128, 512], F32, tag="pe_scratch",
                            name="pe_scratch")
        _warmup_act(nc, consts)
        _warmup_pe(nc, consts, scratch)
        _emit_body(nc, tc, (consts, fwpool, psum, work), drams)
    nc.compile()
    return nc


def build_loop_nc(reps, parts=("dma", "mm", "epi", "out"), fillers=True):
    """Timing variant: run the body `reps` times inside one NEFF (used only
    by the local test harness; the default full-barrier back-edge keeps
    iterations serialized so per-iter span ~ single-shot kernel time)."""
    nc = bacc.Bacc("TRN2", target_bir_lowering=False, debug=False)
    drams = _declare_io(nc)
    with tile.TileContext(nc) as tc, ExitStack() as ctx:
        consts, fwpool, psum, work = _build_pools(tc, ctx)
        scratch = psum.tile([128, 512], F32, tag="pe_scratch",
                            name="pe_scratch")
        _warmup_act(nc, consts)
        with tc.For_i(0, reps, 1):
            if fillers:
                _warmup_pe(nc, consts, scratch)
            _emit_body(nc, tc, (consts, fwpool, psum, work), drams,
                       parts=parts)
    nc.compile()
    return nc


_CACHE = {}


def _get_nc():
    if "nc" not in _CACHE:
        _CACHE["nc"] = _build_nc()
    return _CACHE["nc"]


def _host_prep(inputs):
    f32 = np.float32
    feature = np.ascontiguousarray(inputs["feature"], dtype=f32)
    muW = np.asarray(inputs["muW"], dtype=f32)
    W = np.asarray(inputs["W"], dtype=f32)
    Z = np.asarray(inputs["Z"], dtype=f32)
    logvarW = np.asarray(inputs["logvarW"], dtype=f32)
    logvarZ = np.asarray(inputs["logvarZ"], dtype=f32)

    wstd = np.sqrt(np.exp(logvarW)).astype(f32)
    zstd = np.sqrt(np.exp(logvarZ)).astype(f32)
    a = ((zstd / wstd).astype(f32) * (W - muW)).astype(f32)

    # Column groups; rho weights negated so psum = -u and exp(psum) = e^-u.
    wpi = np.asarray(inputs["h2pi_w"], dtype=f32).T          # [F, M]
    wrho = -np.asarray(inputs["h2rho_w"], dtype=f32).T       # [F, M]
    wrho[:, 0] = 0.0                                         # folded clamp
    wvar = np.asarray(inputs["h2var_w"], dtype=f32).T        # [F, M]
    wrhod = np.concatenate(
        [wrho, np.stack([muW, a, Z, np.zeros_like(muW)], axis=1)], axis=1)

    b_pi = np.asarray(inputs["h2pi_b"], dtype=f32)
    b_rho = -np.asarray(inputs["h2rho_b"], dtype=f32)
    b_rho[0] = -U0                                           # folded clamp
    b_var = np.asarray(inputs["h2var_b"], dtype=f32)
    blk = np.concatenate(
        [np.ones(128, dtype=f32), b_pi, b_rho, np.zeros(4, dtype=f32),
         b_var]).reshape(1, -1).astype(MM_NP)

    # [F, gw] -> [128(p), KC, gw]
    def wfmt(w):
        return np.ascontiguousarray(
            w.reshape(KC, 128, w.shape[1]).transpose(1, 0, 2), dtype=MM_NP)

    wpi_h, wrhod_h, wvar_h = wfmt(wpi), wfmt(wrhod), wfmt(wvar)

    in_maps = []
    for cr in range(NCORES):
        shard = feature[cr * BC:(cr + 1) * BC]               # [BC, F]
        # ft[p, c, t, b] = shard[t*128+b, c*128+p]
        ft = np.ascontiguousarray(
            shard.reshape(NT, 128, KC, 128).transpose(3, 2, 0, 1),
            dtype=MM_NP)
        in_maps.append({"ft": ft, "wpi": wpi_h, "wrhod": wrhod_h,
                        "wvar": wvar_h, "bias_blk": blk})
    return in_maps


def _postprocess(res, cores):
    outs = []
    for name in ("out_pi", "out_mu", "out_var"):
        # [128, NT*M] partition-major -> [BC, M] rows per core
        full = np.concatenate(
            [np.asarray(res.results[c][name], dtype=np.float32)
             .reshape(128, NT, M).transpose(1, 0, 2).reshape(BC, M)
             for c in cores], axis=0)
        outs.append(np.ascontiguousarray(full))
    return tuple(outs)


def kernel(**inputs):
    nc = _get_nc()
    in_maps = _host_prep(inputs)
    res = run_bass_kernel_spmd(nc, in_maps, list(range(NCORES)))
    return _postprocess(res, list(range(NCORES)))
